# revision 1
# baseline (speedup 1.0000x reference)
"""GAT layer kernel for Trainium2, 8 NeuronCores.

Strategy (dst-sharded, zero collectives):
  - Host: append self-loops, sort edges by dst, split dst space into 8 equal
    ranges (one per core).  Per core, node ids are PERMUTED so the core's own
    dst range occupies rows [0, N/8): phase-1 outputs for those rows then sit
    at core-independent addresses (the NEFF is shared by all cores).
  - Phase 1 (replicated): htab[n, 0:136] = [h(128) | a_src(4) | a_dst(4)]
    = x @ W_ext via PE matmuls (bf16), rows padded to 256 bf16 (512B) so the
    row stride satisfies dma_gather's 256B-alignment rule.
  - Phase 2 (per core): dst windows of 128 nodes; each window's edge list is
    split into chunks of 128 slots.  Slots are segregated by source-id half
    (gidx < 32768 vs >= 32768) so the int16-indexed dma_gather can address
    each half-table; two gathers per window run on separate SWDGE queues.
    Per-edge weights w = exp(leakyrelu(a_src + a_dst)) use a max-free softmax
    (scores are bounded, exp cannot overflow in f32); a_dst is expanded from
    the window's 128 nodes to slots with per-chunk PE matmuls against the
    TRANSPOSED onehot.  Aggregation accumulates, per 128-slot chunk,
        psum[d, 0:128] += onehot[e, d] * (w_e * h_e)
        psum[d, 128:132] += onehot[e, d] * w_e
    Host-precomputed onehot/onehotT stream in as fp8 (1.0 exact).
  - Finalize per window: out = elu(layernorm(num/den + bias) * gamma + beta).
"""

import numpy as np
import ml_dtypes

import concourse.bass as bass
import concourse.bacc as bacc
import concourse.mybir as mybir
import concourse.tile as tile
from concourse import library_config
from concourse.bass_utils import run_bass_kernel_spmd

BF16 = ml_dtypes.bfloat16
FP8 = ml_dtypes.float8_e4m3
F32 = mybir.dt.float32
BF16_DT = mybir.dt.bfloat16
FP8_DT = mybir.dt.float8e4
I32 = mybir.dt.int32
I16 = mybir.dt.int16

P = 128


class Cfg:
    def __init__(self, N=50000, E=1600000, DIN=256, DH=128, H=4, NCORES=8):
        self.N, self.E, self.DIN, self.DH, self.H = N, E, DIN, DH, H
        self.C = DH // H
        self.NCORES = NCORES
        self.ROW = DH + 2 * H               # 136 payload cols
        self.ROW_T = 256                    # stored row (512B stride)
        self.D_PER_CORE = N // NCORES       # 6250
        self.NWIN = (self.D_PER_CORE + P - 1) // P   # 49
        self.G1 = 12                        # node tiles per phase-1 group
        nt = (N + P - 1) // P
        self.NT = ((nt + self.G1 - 1) // self.G1) * self.G1   # 396
        self.NPAD = self.NT * P             # 50688
        self.KD = (DIN + P - 1) // P        # matmul k-chunks (2)
        self.WOUT = 4                       # windows batched per output store
        self.HALF = 32768                   # int16-indexable half-table size

    NEG = 0.2
    LN_EPS = 1e-5
    DEN_EPS = 1e-20


DEFAULT_CFG = Cfg()
_PAD0 = True
_REG_OFF = True
NOPS = 8          # cnts slots per window (>= gather ops per window)


def _wrap16(flat):
    """int16 index list -> dma_gather idxs layout [128, n/16]."""
    n = len(flat)
    a = flat.reshape(n // 16, 16).T          # [16, n/16]
    return np.tile(a, (8, 1))                # replicated to 128 partitions


# --------------------------------------------------------------------------
# Host-side preparation (layout only; all FLOPs on x stay on device)
# --------------------------------------------------------------------------

def host_prep(cfg, x, edge_index, W, att_src, att_dst, bias, ln_gamma, ln_beta):
    N, DIN, DH, H, C = cfg.N, cfg.DIN, cfg.DH, cfg.H, cfg.C
    NC, DPC, NWIN = cfg.NCORES, cfg.D_PER_CORE, cfg.NWIN

    x = np.asarray(x, np.float32)
    W = np.asarray(W, np.float32)
    att_src = np.asarray(att_src, np.float32)
    att_dst = np.asarray(att_dst, np.float32)

    Msrc = np.zeros((DH, H), np.float32)
    Mdst = np.zeros((DH, H), np.float32)
    for h in range(H):
        Msrc[h * C:(h + 1) * C, h] = att_src[h]
        Mdst[h * C:(h + 1) * C, h] = att_dst[h]
    W_ext = np.concatenate([W, W @ Msrc, W @ Mdst], axis=1)  # [DIN, ROW]
    W16 = np.ascontiguousarray(W_ext).astype(BF16)

    # edges sorted by dst (self-loops handled on device via identity chunk)
    src = np.asarray(edge_index[0], np.int64)
    dst = np.asarray(edge_index[1], np.int64)
    order = np.argsort(dst, kind="stable")
    src_s = src[order].astype(np.int64)
    dst_s = dst[order].astype(np.int64)

    grid = (np.arange(NC)[:, None] * DPC
            + np.minimum(np.arange(NWIN) * P, DPC)[None, :]).ravel()
    eb = np.searchsorted(dst_s, grid).astype(np.int64)
    eb = np.append(eb, len(dst_s)).reshape(1, -1)
    e_start = eb.ravel()[:-1].reshape(NC, NWIN)
    e_end = np.append(e_start.ravel()[1:], len(dst_s)).reshape(NC, NWIN)

    # per (core, window, half) counts -> global K0/K1
    cnt0 = np.zeros((NC, NWIN), np.int64)
    cnt1 = np.zeros((NC, NWIN), np.int64)
    gidx_all = np.empty(len(src_s), np.int64)
    for c in range(NC):
        base = c * DPC
        # permutation: own dst range first, then the rest in order
        # gidx(n) = n - base if base <= n < base+DPC else
        #           n + DPC if n < base else n
        s = src_s
        g = np.where((s >= base) & (s < base + DPC), s - base,
                     np.where(s < base, s + DPC, s))
        lo, hi = int(e_start[c, 0]), int(e_end[c, -1])
        gidx_all[lo:hi] = g[lo:hi]
        h0 = g[lo:hi] < cfg.HALF
        w_of_e = (dst_s[lo:hi] - base) >> 7
        np.add.at(cnt0[c], w_of_e, h0)
        np.add.at(cnt1[c], w_of_e, ~h0)
    K0 = int(np.ceil(cnt0.max() / P))
    K1 = int(np.ceil(cnt1.max() / P))
    KW = K0 + K1
    # static per-window gather counts: max over cores (NEFF is shared across
    # cores, not windows), rounded up to the 16-index vector granularity
    nidx0 = tuple(int(min(((v + 15) // 16) * 16, K0 * P))
                  for v in cnt0.max(axis=0))
    nidx1 = tuple(int(min(((v + 15) // 16) * 16, K1 * P))
                  for v in cnt1.max(axis=0))

    gbb = np.stack([np.asarray(ln_gamma, np.float32),
                    np.asarray(ln_beta, np.float32),
                    np.asarray(bias, np.float32)], 0)

    in_maps = []
    for c in range(NC):
        base = c * DPC
        # permuted xT for this core
        perm = np.concatenate([np.arange(base, base + DPC),
                               np.arange(0, base),
                               np.arange(base + DPC, N)])
        xTp = np.zeros((DIN, cfg.NPAD), np.float32)
        xTp[:, :N] = x.T[:, perm]
        lo, hi = int(e_start[c, 0]), int(e_end[c, -1])
        g = gidx_all[lo:hi]
        edl = (dst_s[lo:hi] - base).astype(np.int64)   # local dst [0, DPC)
        w_of_e = edl >> 7
        h0 = g < cfg.HALF
        # slot position: within (window, half) running index
        pos = np.zeros(hi - lo, np.int64)
        for wv in range(NWIN):
            m = w_of_e == wv
            m0 = m & h0
            m1 = m & ~h0
            pos[m0] = np.arange(m0.sum())
            pos[m1] = K0 * P + np.arange(m1.sum())
        slot = (w_of_e * KW + pos // P) * P + (pos & 127)

        nslots = NWIN * KW * P
        # pad -> -1: trailing negatives make the Q7 desc-gen trim them
        # (pads are a suffix of each (window, half) region, hence of each op)
        flat_g = np.full(nslots, 0 if _PAD0 else -1, np.int64)
        flat_g[slot] = np.where(h0, g, g - cfg.HALF)
        oh = np.zeros((nslots, P), np.uint8)
        oh[slot, edl & 127] = 1

        # int16 idx tensor [NWIN, 128, KW*8]
        si16 = np.zeros((NWIN, P, KW * 8), np.int16)
        fg = flat_g.reshape(NWIN, KW * P).astype(np.int16)
        # pads must be a suffix of every gather op's index list (ops split
        # each half at chunk granularity, reals fill the region prefix)
        for wv in range(NWIN):
            for lo_, hi_ in ((0, K0 * P), (K0 * P, KW * P)):
                seg = fg[wv, lo_:hi_]
                neg = np.flatnonzero(seg < 0)
                if len(neg):
                    assert (seg[neg[0]:] < 0).all(), (c, wv, lo_)
        for wv in range(NWIN):
            if K0:
                si16[wv, :, :K0 * 8] = _wrap16(fg[wv, :K0 * P])
            if K1:
                si16[wv, :, K0 * 8:] = _wrap16(fg[wv, K0 * P:])

        oh4 = oh.reshape(NWIN, KW, P, P)
        ohdev = np.ascontiguousarray(
            oh4.transpose(0, 2, 1, 3)).reshape(NWIN, P, KW * P).astype(FP8)
        ohT = np.ascontiguousarray(
            oh4.transpose(0, 3, 1, 2)).reshape(NWIN, P, KW * P).astype(FP8)

        # exact per-op index counts (16-granular) for num_idxs_reg
        ops = ([(k, min(k + 8, K0)) for k in range(0, K0, 8)]
               + [(K0 + k, K0 + min(k + 8, K1)) for k in range(0, K1, 8)])
        cnts = np.zeros((NWIN, NOPS), np.int32)
        for j, (a, b) in enumerate(ops):
            span = (b - a) * P
            rel = (a - K0) * P if a >= K0 else a * P
            base_cnt = cnt1[c] if a >= K0 else cnt0[c]
            cv = np.clip(base_cnt - rel, 0, span)
            cnts[:, j] = np.maximum(((cv + 15) // 16) * 16, 16).astype(np.int32)

        in_maps.append({
            "xT": xTp.astype(BF16),
            "wext": W16,
            "si16": np.ascontiguousarray(si16.transpose(1, 0, 2)),
            "onehot": ohdev,
            "onehotT": ohT,
            "gbb": gbb,
            "ident": np.eye(P, dtype=FP8),
        })
    return in_maps, K0, K1, nidx0, nidx1



def _gathers(nc, g_main, htab, si_t, w, k_base, nidx, tab_lo, tab_hi, sub=8):
    """dma_gather htab[tab_lo:tab_hi] rows into g_main chunks starting at
    k_base, nidx indices total (static, 16-granular; the tail op may cover a
    partial chunk).  Ops are split at <=sub chunks (ring-pressure limit)."""
    kn = (nidx + P - 1) // P
    k = 0
    while k < kn:
        ke = min(k + sub, kn)
        ni = min((ke - k) * P, nidx - k * P)
        nc.gpsimd.dma_gather(
            out_ap=g_main[:, k_base + k:k_base + ke, :],
            in_ap=htab[tab_lo:tab_hi, :],
            idxs_ap=si_t[:, w, (k_base + k) * 8:(k_base + ke) * 8],
            num_idxs=ni, num_idxs_reg=ni, elem_size=g_main.shape[-1])
        k = ke


# --------------------------------------------------------------------------
# Bass kernel builder (identical NEFF for all cores)
# --------------------------------------------------------------------------

def build_nc(cfg, K0, K1, nidx0, nidx1, dbg=None):
    N, DIN, DH, H, C = cfg.N, cfg.DIN, cfg.DH, cfg.H, cfg.C
    ROW, ROW_T, NWIN, NT, NPAD, G1, KD = (cfg.ROW, cfg.ROW_T, cfg.NWIN,
                                          cfg.NT, cfg.NPAD, cfg.G1, cfg.KD)
    KW = K0 + K1
    NG1 = NT // G1
    BPG = (G1 + 2) // 3
    WOUT = cfg.WOUT

    HALF = cfg.HALF
    nc = bacc.Bacc("TRN2")
    xT_d = nc.dram_tensor("xT", [DIN, NPAD], BF16_DT, kind="ExternalInput")
    w_d = nc.dram_tensor("wext", [DIN, ROW], BF16_DT, kind="ExternalInput")
    si_d = nc.dram_tensor("si16", [P, NWIN, KW * 8], I16, kind="ExternalInput")
    oh_d = nc.dram_tensor("onehot", [NWIN, P, KW * P], FP8_DT,
                          kind="ExternalInput")
    ohT_d = nc.dram_tensor("onehotT", [NWIN, P, KW * P], FP8_DT,
                           kind="ExternalInput")
    gbb_d = nc.dram_tensor("gbb", [3, DH], F32, kind="ExternalInput")
    id_d = nc.dram_tensor("ident", [P, P], FP8_DT, kind="ExternalInput")
    y_d = nc.dram_tensor("y", [NWIN * P, DH], F32, kind="ExternalOutput")
    htab0 = nc.dram_tensor("htab0", [HALF, ROW_T], BF16_DT, kind="Internal")
    htab1 = nc.dram_tensor("htab1", [NPAD - HALF, ROW_T], BF16_DT,
                           kind="Internal")

    MPBUFS = 4
    nc.gpsimd.load_library(library_config.mlp)
    with tile.TileContext(nc) as tc:
        with tc.tile_pool(name="const", bufs=1) as const, \
             tc.tile_pool(name="mp", bufs=MPBUFS) as mp:
            wt = const.tile([P, KD, ROW], BF16_DT)
            for k in range(KD):
                nc.sync.dma_start(out=wt[:, k, :], in_=w_d[k * P:(k + 1) * P, :])
            si_t = const.tile([P, NWIN, KW * 8], I16)
            nc.sync.dma_start(out=si_t[:], in_=si_d[:])
            gam_t = const.tile([P, DH], F32)
            bet_t = const.tile([P, DH], F32)
            bia_t = const.tile([P, DH], F32)
            for t, i in ((gam_t, 0), (bet_t, 1), (bia_t, 2)):
                a = gbb_d[i, :]
                src_ap = bass.AP(a.tensor, a.offset, [[0, P], [1, DH]])
                nc.gpsimd.dma_start(out=t[:], in_=src_ap)
            eps_t = const.tile([P, 1], F32)
            nc.vector.memset(eps_t[:], cfg.LN_EPS)
            id_t = const.tile([P, P], FP8_DT)
            nc.sync.dma_start(out=id_t[:], in_=id_d[:])

            # ---- phase 1: htab[:, 0:136] = x @ W_ext ----
            for _ in range(MPBUFS):
                gz = mp.tile([P, KW, ROW_T], BF16_DT, tag="gm")
                nc.vector.memset(gz[:], 0.0)
            with (
                tc.tile_pool(name="xp", bufs=2) as xp,
                tc.tile_pool(name="stg", bufs=2) as stg,
                tc.tile_pool(name="ps1", bufs=2, space="PSUM") as ps1,
            ):
                for g in range(NG1):
                    xk = xp.tile([P, KD, G1 * P], BF16_DT)
                    for k in range(KD):
                        nc.sync.dma_start(
                            out=xk[:, k, :],
                            in_=xT_d[k * P:(k + 1) * P,
                                     g * G1 * P:(g + 1) * G1 * P])
                    ps = ps1.tile([P, BPG, 512], F32, tag="ps1")
                    for i in range(G1):
                        pslice = ps[:, i // 3, (i % 3) * ROW:(i % 3 + 1) * ROW]
                        for k in range(KD):
                            nc.tensor.matmul(
                                pslice, lhsT=xk[:, k, i * P:(i + 1) * P],
                                rhs=wt[:, k, :],
                                start=(k == 0), stop=(k == KD - 1))
                    stage = stg.tile([P, G1, ROW], BF16_DT, tag="stage")
                    nc.scalar.copy(
                        out=stage[:].rearrange("p (b t) r -> p b t r", t=3),
                        in_=ps[:, :, 0:3 * ROW].rearrange(
                            "p b (t r) -> p b t r", r=ROW))
                    n0 = g * G1 * P
                    n1 = n0 + G1 * P
                    if n1 <= HALF or n0 >= HALF:
                        t, o = (htab0, n0) if n1 <= HALF else (htab1, n0 - HALF)
                        dst_ap = t[o:o + G1 * P, 0:ROW].rearrange(
                            "(b p) r -> p b r", p=P)
                        nc.gpsimd.dma_start(out=dst_ap, in_=stage[:])
                    else:
                        bs = (HALF - n0) // P   # boundary is 128-aligned
                        dst_ap = htab0[n0:HALF, 0:ROW].rearrange(
                            "(b p) r -> p b r", p=P)
                        nc.gpsimd.dma_start(out=dst_ap, in_=stage[:, 0:bs, :])
                        dst_ap = htab1[0:n1 - HALF, 0:ROW].rearrange(
                            "(b p) r -> p b r", p=P)
                        nc.gpsimd.dma_start(out=dst_ap, in_=stage[:, bs:, :])

            if dbg == "phase1":
                with tc.tile_pool(name="dbgp", bufs=2) as dbgp:
                    for w in range(NWIN):
                        t = dbgp.tile([P, DH], F32, tag="dbg")
                        nc.gpsimd.dma_start(out=t[:],
                                            in_=htab0[w * P:(w + 1) * P, 0:DH])
                        nc.gpsimd.dma_start(out=y_d[w * P:(w + 1) * P, :],
                                            in_=t[:])
            if dbg == "gather":
                with (
                    tc.tile_pool(name="gmp", bufs=2) as gmp,
                    tc.tile_pool(name="gfp", bufs=2) as gfp,
                ):
                    for w in range(NWIN):
                        g_main = gmp.tile([P, KW, ROW_T], BF16_DT, tag="gm")
                        _gathers(nc, g_main, htab0, si_t, w, 0, K0, 0, HALF)
                        _gathers(nc, g_main, htab1, si_t, w, K0, KW,
                                 0, NPAD - HALF)
                        t = gfp.tile([P, DH], F32, tag="dbg")
                        nc.vector.tensor_copy(out=t[:], in_=g_main[:, 0, 0:DH])
                        nc.gpsimd.dma_start(out=y_d[w * P:(w + 1) * P, :],
                                            in_=t[:])
            if dbg is None:
              # ---- phase 2 ----
              with (
                  tc.tile_pool(name="op", bufs=2) as op,
                  tc.tile_pool(name="otp", bufs=2) as otp,
                  tc.tile_pool(name="hwp", bufs=2) as hwp,
                  tc.tile_pool(name="rp", bufs=2) as rp,
                  tc.tile_pool(name="wp", bufs=2) as wp,
                  tc.tile_pool(name="ps2", bufs=2, space="PSUM") as ps2,
                  tc.tile_pool(name="pse", bufs=2, space="PSUM") as pse,
                  tc.tile_pool(name="fp", bufs=2) as fp,
                  tc.tile_pool(name="outp", bufs=2) as outp,
              ):
                  # a_dst for this core's 6272 dst rows (= permuted rows 0..)
                  adw_t = const.tile([P, NWIN, H], BF16_DT)
                  nc.scalar.dma_start(
                      out=adw_t[:],
                      in_=htab0[0:NWIN * P, DH + H:DH + 2 * H].rearrange(
                          "(w p) r -> p w r", p=P))
                  ost = None
                  for w in range(NWIN):
                      g_main = mp.tile([P, KW, ROW_T], BF16_DT, tag="gm")
                      _gathers(nc, g_main, htab0, si_t, w, 0, nidx0[w],
                               0, HALF)
                      _gathers(nc, g_main, htab1, si_t, w, K0, nidx1[w],
                               0, NPAD - HALF)
                      ks = (list(range((nidx0[w] + P - 1) // P))
                            + [K0 + kk
                               for kk in range((nidx1[w] + P - 1) // P)])
                      oh_t = op.tile([P, KW * P], FP8_DT, tag="oh")
                      nc.sync.dma_start(out=oh_t[:], in_=oh_d[w])
                      ohT_t = otp.tile([P, KW * P], FP8_DT, tag="ohT")
                      nc.sync.dma_start(out=ohT_t[:], in_=ohT_d[w])
                      # own dst rows (dense): self-loop messages need no gather
                      hw_t = hwp.tile([P, ROW], BF16_DT, tag="hw")
                      nc.scalar.dma_start(out=hw_t[:],
                                          in_=htab0[w * P:(w + 1) * P, 0:ROW])

                      # a_dst expansion: dpx[slot, h] per chunk via PE
                      pe = pse.tile([P, KW * H], F32, tag="pse")
                      for k in ks:
                          nc.tensor.matmul(pe[:, k * H:(k + 1) * H],
                                           lhsT=ohT_t[:, k * P:(k + 1) * P],
                                           rhs=adw_t[:, w, :],
                                           start=True, stop=True)
                      dpx = wp.tile([P, KW, H], BF16_DT, tag="dpx")
                      nc.scalar.copy(out=dpx[:], in_=pe[:].rearrange(
                          "p (k h) -> p k h", h=H))

                      sc = wp.tile([P, KW, H], F32, tag="sc")
                      nc.vector.tensor_tensor(
                          out=sc[:], in0=g_main[:, :, DH:DH + H],
                          in1=dpx[:], op=mybir.AluOpType.add)
                      sc2 = wp.tile([P, KW, H], F32, tag="sc2")
                      nc.vector.tensor_scalar_mul(out=sc2[:], in0=sc[:],
                                                  scalar1=cfg.NEG)
                      nc.vector.tensor_tensor(out=sc2[:], in0=sc[:], in1=sc2[:],
                                              op=mybir.AluOpType.max)
                      wf = wp.tile([P, KW, H], BF16_DT, tag="wf")
                      nc.scalar.activation(out=wf[:], in_=sc2[:],
                                           func=mybir.ActivationFunctionType.Exp)

                      rhs = rp.tile([P, KW, ROW - H], BF16_DT, tag="rhs")
                      nc.vector.tensor_copy(out=rhs[:, :, DH:DH + H], in_=wf[:])
                      a = wf[:]
                      w_bcast = bass.AP(a.tensor, a.offset,
                                        [a.ap[0], a.ap[1], a.ap[2], [0, C]])
                      nc.vector.tensor_tensor(
                          out=rhs[:, :, 0:DH].rearrange("p k (h c) -> p k h c",
                                                        h=H),
                          in0=g_main[:, :, 0:DH].rearrange("p k (h c) -> p k h c",
                                                           h=H),
                          in1=w_bcast, op=mybir.AluOpType.mult)

                      # self-loop scores/messages from the dense own-row tile
                      scs = wp.tile([P, H], F32, tag="scs")
                      nc.vector.tensor_tensor(out=scs[:],
                                              in0=hw_t[:, DH:DH + H],
                                              in1=hw_t[:, DH + H:DH + 2 * H],
                                              op=mybir.AluOpType.add)
                      scs2 = wp.tile([P, H], F32, tag="scs2")
                      nc.vector.tensor_scalar_mul(out=scs2[:], in0=scs[:],
                                                  scalar1=cfg.NEG)
                      nc.vector.tensor_tensor(out=scs2[:], in0=scs[:],
                                              in1=scs2[:],
                                              op=mybir.AluOpType.max)
                      wfs = wp.tile([P, H], BF16_DT, tag="wfs")
                      nc.scalar.activation(out=wfs[:], in_=scs2[:],
                                           func=mybir.ActivationFunctionType.Exp)
                      rhs_s = rp.tile([P, ROW - H], BF16_DT, tag="rhss")
                      nc.vector.tensor_copy(out=rhs_s[:, DH:DH + H], in_=wfs[:])
                      a2 = wfs[:]
                      wfs_b = bass.AP(a2.tensor, a2.offset,
                                      [a2.ap[0], a2.ap[1], [0, C]])
                      nc.vector.tensor_tensor(
                          out=rhs_s[:, 0:DH].rearrange("p (h c) -> p h c", h=H),
                          in0=hw_t[:, 0:DH].rearrange("p (h c) -> p h c", h=H),
                          in1=wfs_b, op=mybir.AluOpType.mult)

                      ps = ps2.tile([P, DH + H], F32, tag="psw")
                      for i, k in enumerate(ks):
                          nc.tensor.matmul(ps[:],
                                           lhsT=oh_t[:, k * P:(k + 1) * P],
                                           rhs=rhs[:, k, :],
                                           start=(i == 0), stop=False)
                      nc.tensor.matmul(ps[:], lhsT=id_t[:], rhs=rhs_s[:],
                                       start=False, stop=True)

                      den = fp.tile([P, H], F32, tag="den")
                      nc.vector.tensor_scalar_add(out=den[:],
                                                  in0=ps[:, DH:DH + H],
                                                  scalar1=cfg.DEN_EPS)
                      nc.vector.reciprocal(out=den[:], in_=den[:])
                      y = fp.tile([P, DH], F32, tag="y")
                      da = den[:]
                      den_bcast = bass.AP(da.tensor, da.offset,
                                          [da.ap[0], da.ap[1], [0, C]])
                      nc.vector.tensor_tensor(
                          out=y[:].rearrange("p (h c) -> p h c", h=H),
                          in0=ps[:, 0:DH].rearrange("p (h c) -> p h c", h=H),
                          in1=den_bcast, op=mybir.AluOpType.mult)
                      nc.vector.tensor_tensor(out=y[:], in0=y[:], in1=bia_t[:],
                                              op=mybir.AluOpType.add)
                      st = fp.tile([P, 6], F32, tag="st")
                      nc.vector.bn_stats(out=st[:], in_=y[:])
                      mv = fp.tile([P, 2], F32, tag="mv")
                      nc.vector.bn_aggr(out=mv[:], in_=st[:])
                      nc.scalar.activation(out=mv[:, 1:2], in_=mv[:, 1:2],
                                           func=mybir.ActivationFunctionType.Sqrt,
                                           bias=eps_t[:])
                      nc.vector.reciprocal(out=mv[:, 1:2], in_=mv[:, 1:2])
                      z = fp.tile([P, DH], F32, tag="z")
                      nc.vector.tensor_scalar(out=z[:], in0=y[:],
                                              scalar1=mv[:, 0:1],
                                              scalar2=mv[:, 1:2],
                                              op0=mybir.AluOpType.subtract,
                                              op1=mybir.AluOpType.mult)
                      nc.vector.tensor_tensor(out=z[:], in0=z[:], in1=gam_t[:],
                                              op=mybir.AluOpType.mult)
                      nc.vector.tensor_tensor(out=z[:], in0=z[:], in1=bet_t[:],
                                              op=mybir.AluOpType.add)
                      zm = fp.tile([P, DH], F32, tag="zm")
                      nc.vector.tensor_scalar(out=zm[:], in0=z[:],
                                              scalar1=0.0, scalar2=-1.0,
                                              op0=mybir.AluOpType.max,
                                              op1=mybir.AluOpType.add)
                      zn = fp.tile([P, DH], F32, tag="zn")
                      nc.vector.tensor_scalar(out=zn[:], in0=z[:], scalar1=0.0,
                                              scalar2=None,
                                              op0=mybir.AluOpType.min)
                      te = fp.tile([P, DH], F32, tag="te")
                      nc.scalar.activation(out=te[:], in_=zn[:],
                                           func=mybir.ActivationFunctionType.Exp)
                      if w % WOUT == 0:
                          ost = outp.tile([P, WOUT, DH], F32, tag="ost")
                      nc.vector.tensor_tensor(out=ost[:, w % WOUT, :], in0=zm[:],
                                              in1=te[:], op=mybir.AluOpType.add)
                      if w % WOUT == WOUT - 1 or w == NWIN - 1:
                          w0 = (w // WOUT) * WOUT
                          nb = w - w0 + 1
                          dst_ap = y_d[w0 * P:(w + 1) * P, :].rearrange(
                              "(b p) r -> p b r", p=P)
                          nc.scalar.dma_start(out=dst_ap, in_=ost[:, :nb, :])

    nc.compile()
    return nc


# --------------------------------------------------------------------------
# Entry point
# --------------------------------------------------------------------------

_CACHE = {}


def kernel(x, edge_index, W, att_src, att_dst, bias, ln_gamma, ln_beta,
           cfg=DEFAULT_CFG, trace=False, dbg=None):
    in_maps, K0, K1, nidx0, nidx1 = host_prep(cfg, x, edge_index, W,
                                              att_src, att_dst,
                                              bias, ln_gamma, ln_beta)
    key = (cfg.N, cfg.E, K0, K1, nidx0, nidx1, dbg)
    if key not in _CACHE:
        _CACHE[key] = build_nc(cfg, K0, K1, nidx0, nidx1, dbg=dbg)
    nc = _CACHE[key]
    r = run_bass_kernel_spmd(nc, in_maps, core_ids=list(range(cfg.NCORES)),
                             trace=trace)
    out = np.empty((cfg.N, cfg.DH), np.float32)
    for c in range(cfg.NCORES):
        out[c * cfg.D_PER_CORE:(c + 1) * cfg.D_PER_CORE] = \
            r.results[c]["y"][:cfg.D_PER_CORE]
    kernel.last_result = r
    return out



# revision 11
# speedup vs baseline: 1.1204x; 1.1204x over previous
"""GAT layer kernel for Trainium2, 8 NeuronCores.

Strategy (dst-sharded, zero collectives, identity-slot aggregation):
  - Host: append self-loops, split dst space into 8 equal ranges (one per
    core).  Per core, dst nodes are RELABELED in decreasing (in-degree+1)
    order; window w = labels [128w, 128w+128).  Edge (k-th incoming edge of
    label p, half hf) occupies gather slot (chunk, partition=p) where chunk
    enumerates k within the half-region.  Aggregation over a node's edges is
    then a plain sum over chunks at fixed partition -- NO onehot matmuls.
  - Phase 1 (replicated): htab[n] = [h(128) | a_src(4) | a_dst(4)] = x @
    W_ext via PE matmuls (bf16), rows strided 512B (dma_gather needs 256B
    multiples).  Tables split at S0=32512 so int16 gather indices reach every
    row; one sentinel row per table (h=0, a_src=-100) absorbs pad slots.
  - Phase 2 (per core, per window): one dma_gather per (window, half) on
    rotating SWDGE queues fetches rows into g[d-part, chunk, 512B].  Scores
    w = exp(leakyrelu(a_src + a_dst)) use the max-free softmax (bounded
    scores).  a_dst comes from the self-loop slot (chunk 0 of the node's own
    half).  num = sum_k w*h and den = sum_k w are DVE reduces; finalize is
    out = elu(layernorm(num/den + bias) * gamma + beta).  Output rows are in
    degree-sorted order; the host unpermutes.
"""

import numpy as np
import ml_dtypes

import concourse.bass as bass
import concourse.bacc as bacc
import concourse.mybir as mybir
import concourse.tile as tile
from concourse import library_config
from concourse.bass_utils import run_bass_kernel_spmd

BF16 = ml_dtypes.bfloat16
F32 = mybir.dt.float32
BF16_DT = mybir.dt.bfloat16
I16 = mybir.dt.int16

P = 128


class Cfg:
    def __init__(self, N=50000, E=1600000, DIN=256, DH=128, H=4, NCORES=8):
        self.N, self.E, self.DIN, self.DH, self.H = N, E, DIN, DH, H
        self.C = DH // H
        self.NCORES = NCORES
        self.ROW = DH + 2 * H               # 136 payload cols
        self.ROW_T = 256                    # stored row (512B stride)
        self.D_PER_CORE = N // NCORES       # 6250
        self.NWIN = (self.D_PER_CORE + P - 1) // P   # 49
        self.G1 = 12                        # node tiles per phase-1 group
        nt = (N + P - 1) // P
        self.NT = ((nt + self.G1 - 1) // self.G1) * self.G1   # 396
        self.NPAD = self.NT * P             # 50688
        self.KD = (DIN + P - 1) // P        # matmul k-chunks (2)
        self.WOUT = 4                       # windows batched per output store
        self.S0 = 32512                     # htab0 rows (254*128, idx<=32512)
        self.GSUB = 8                       # max chunks per gather op

    NEG = 0.2
    LN_EPS = 1e-5
    DEN_EPS = 1e-20
    SENT_A = -100.0


DEFAULT_CFG = Cfg()


def _wrap16(flat):
    """int16 index list -> dma_gather idxs layout [128, n/16]."""
    n = len(flat)
    a = flat.reshape(n // 16, 16).T          # [16, n/16]
    return np.tile(a, (8, 1))                # replicated to 128 partitions


# --------------------------------------------------------------------------
# Host-side preparation (layout only; all FLOPs on x stay on device)
# --------------------------------------------------------------------------

def host_prep(cfg, x, edge_index, W, att_src, att_dst, bias, ln_gamma, ln_beta):
    N, DIN, DH, H, C = cfg.N, cfg.DIN, cfg.DH, cfg.H, cfg.C
    NC, DPC, NWIN, S0 = cfg.NCORES, cfg.D_PER_CORE, cfg.NWIN, cfg.S0
    N1 = cfg.NPAD - S0                       # htab1 node rows

    x = np.asarray(x, np.float32)
    W = np.asarray(W, np.float32)
    att_src = np.asarray(att_src, np.float32)
    att_dst = np.asarray(att_dst, np.float32)

    Msrc = np.zeros((DH, H), np.float32)
    Mdst = np.zeros((DH, H), np.float32)
    for h in range(H):
        Msrc[h * C:(h + 1) * C, h] = att_src[h]
        Mdst[h * C:(h + 1) * C, h] = att_dst[h]
    W_ext = np.concatenate([W, W @ Msrc, W @ Mdst], axis=1)  # [DIN, ROW]
    W16 = np.ascontiguousarray(W_ext).astype(BF16)

    xT = np.zeros((DIN, cfg.NPAD), np.float32)
    xT[:, :N] = x.T
    xT16 = xT.astype(BF16)

    src = np.asarray(edge_index[0], np.int64)
    dst = np.asarray(edge_index[1], np.int64)

    gbb = np.stack([np.asarray(ln_gamma, np.float32),
                    np.asarray(ln_beta, np.float32),
                    np.asarray(bias, np.float32)], 0)
    # sentinel rows: h = 0, a_src = SENT_A, a_dst = 0
    sent = np.zeros((2, cfg.ROW), np.float32)
    sent[:, DH:DH + H] = cfg.SENT_A
    sent16 = sent.astype(BF16)

    # ---- per-core edge layout ----
    core_of = dst // DPC
    per_core = []   # (order, c0, c1) per core
    for c in range(NC):
        m = core_of == c
        s_c = src[m]
        d_loc = dst[m] - c * DPC
        # self-loops for this core's dst range
        own = np.arange(c * DPC, (c + 1) * DPC, dtype=np.int64)
        s_all = np.concatenate([own, s_c])
        d_all = np.concatenate([own - c * DPC, d_loc])
        is_self = np.zeros(len(s_all), np.int8)
        is_self[:DPC] = 1      # used to force self-loop to slot k=0
        hf = (s_all >= S0).astype(np.int64)
        c0 = np.bincount(d_all[hf == 0], minlength=DPC)
        c1 = np.bincount(d_all[hf == 1], minlength=DPC)
        # band by c0, sort by c1 within band: windows then have both a tight
        # max-c0 (band width) and a tight max-c1 (sorted runs)
        order = np.lexsort((-c1, -(c0 // 6)))          # node ids, label order
        per_core.append((order, c0, c1, s_all, d_all, hf, is_self))

    # static per-window chunk counts: max over cores
    K0s = np.zeros(NWIN, np.int64)
    K1s = np.zeros(NWIN, np.int64)
    for c in range(NC):
        order, c0, c1 = per_core[c][0], per_core[c][1], per_core[c][2]
        c0s = np.zeros(NWIN * P, np.int64)
        c1s = np.zeros(NWIN * P, np.int64)
        c0s[:DPC] = c0[order]
        c1s[:DPC] = c1[order]
        K0s = np.maximum(K0s, c0s.reshape(NWIN, P).max(1))
        K1s = np.maximum(K1s, c1s.reshape(NWIN, P).max(1))
    KWs = K0s + K1s
    offs = np.zeros(NWIN + 1, np.int64)
    np.cumsum(KWs, out=offs[1:])
    TOTCH = int(offs[-1])

    in_maps = []
    out_perms = []
    for c in range(NC):
        order, c0, c1, s_all, d_all, hf, is_self = per_core[c]
        label_of = np.empty(DPC, np.int64)
        label_of[order] = np.arange(DPC)
        lab = label_of[d_all]                      # label per edge
        w_of = lab >> 7
        p_of = lab & 127
        # position of each edge within its (label, half) group, self first
        key = ((lab * 2 + hf) << 1) | (1 - is_self).astype(np.int64)
        eo = np.argsort(key, kind="stable")
        ks = key[eo] >> 1                          # group id = lab*2+hf
        starts = np.searchsorted(ks, np.arange(DPC * 2) * 1)
        # position within group
        grp_start = starts[ks]
        pos = np.arange(len(eo)) - grp_start
        # chunk index within window
        k0w = K0s[w_of[eo]]
        chunk = np.where(hf[eo] == 0, pos, k0w + pos)
        gchunk = offs[w_of[eo]] + chunk
        slot = gchunk * P + p_of[eo]
        rowid = np.where(hf[eo] == 0, s_all[eo], s_all[eo] - S0)

        flat = np.empty(TOTCH * P, np.int16)
        # defaults: sentinel of the chunk's half
        half1_chunk = np.zeros(TOTCH, bool)
        for w in range(NWIN):
            half1_chunk[offs[w] + K0s[w]:offs[w + 1]] = True
        flat.reshape(TOTCH, P)[~half1_chunk] = S0        # sentinel htab0
        flat.reshape(TOTCH, P)[half1_chunk] = N1         # sentinel htab1
        flat[slot] = rowid.astype(np.int16)

        si16 = np.zeros((P, TOTCH * 8), np.int16)
        for w in range(NWIN):
            a, b = int(offs[w]), int(offs[w] + K0s[w])
            if b > a:
                si16[:, a * 8:b * 8] = _wrap16(flat[a * P:b * P])
            a, b = int(offs[w] + K0s[w]), int(offs[w + 1])
            if b > a:
                si16[:, a * 8:b * 8] = _wrap16(flat[a * P:b * P])

        # window-node half masks (m0=1.0 if node id < S0 -> self-loop slot in
        # half0; m1 = 1 - m0).  Pad labels (>= DPC) point at sentinel rows
        # whose a_dst is 0 -- either half works.
        node_of_label = np.zeros(NWIN * P, np.int64)
        node_of_label[:DPC] = order + c * DPC
        glob = node_of_label.reshape(NWIN, P).T       # [P, NWIN]
        m0h = (glob < S0).astype(np.float32)
        m0 = np.stack([m0h, 1.0 - m0h], axis=2)       # [P, NWIN, 2]

        in_maps.append({
            "xT": xT16,
            "wext": W16,
            "si16": si16,
            "m0": np.ascontiguousarray(m0.reshape(P, 2 * NWIN)),
            "gbb": gbb,
            "sent": sent16,
        })
        out_perms.append(order + c * DPC)
    return in_maps, tuple(int(v) for v in K0s), tuple(int(v) for v in K1s), \
        TOTCH, out_perms


# --------------------------------------------------------------------------
# Bass kernel builder (identical NEFF for all cores)
# --------------------------------------------------------------------------

def build_nc(cfg, K0s, K1s, TOTCH, dbg=None):
    N, DIN, DH, H, C = cfg.N, cfg.DIN, cfg.DH, cfg.H, cfg.C
    ROW, ROW_T, NWIN, NT, NPAD, G1, KD = (cfg.ROW, cfg.ROW_T, cfg.NWIN,
                                          cfg.NT, cfg.NPAD, cfg.G1, cfg.KD)
    S0 = cfg.S0
    N1 = NPAD - S0
    KWs = [a + b for a, b in zip(K0s, K1s)]
    KWMAX = max(KWs)
    offs = [0]
    for v in KWs:
        offs.append(offs[-1] + v)
    NG1 = NT // G1
    BPG = (G1 + 2) // 3
    WOUT = cfg.WOUT

    nc = bacc.Bacc("TRN2", num_swdge_queues=4)
    xT_d = nc.dram_tensor("xT", [DIN, NPAD], BF16_DT, kind="ExternalInput")
    w_d = nc.dram_tensor("wext", [DIN, ROW], BF16_DT, kind="ExternalInput")
    si_d = nc.dram_tensor("si16", [P, TOTCH * 8], I16, kind="ExternalInput")
    m0_d = nc.dram_tensor("m0", [P, 2 * NWIN], F32, kind="ExternalInput")
    gbb_d = nc.dram_tensor("gbb", [3, DH], F32, kind="ExternalInput")
    sent_d = nc.dram_tensor("sent", [2, ROW], BF16_DT, kind="ExternalInput")
    y_d = nc.dram_tensor("y", [NWIN * P, DH], F32, kind="ExternalOutput")
    htab0 = nc.dram_tensor("htab0", [S0 + P, ROW_T], BF16_DT, kind="Internal")
    htab1 = nc.dram_tensor("htab1", [N1 + P, ROW_T], BF16_DT, kind="Internal")

    qrr = [0]

    def next_q():
        q = qrr[0]
        qrr[0] = (q + 1) % 4
        return q

    nc.gpsimd.load_library(library_config.mlp)
    with tile.TileContext(nc) as tc:
        with tc.tile_pool(name="const", bufs=1) as const, \
             tc.tile_pool(name="mp", bufs=2) as mp:
            wt = const.tile([P, KD, ROW], BF16_DT)
            for k in range(KD):
                nc.sync.dma_start(out=wt[:, k, :], in_=w_d[k * P:(k + 1) * P, :])
            si_t = const.tile([P, TOTCH * 8], I16)
            nc.sync.dma_start(out=si_t[:], in_=si_d[:])
            m0_t = const.tile([P, 2 * NWIN], F32)
            nc.sync.dma_start(out=m0_t[:], in_=m0_d[:])
            gam_t = const.tile([P, DH], F32)
            bet_t = const.tile([P, DH], F32)
            bia_t = const.tile([P, DH], F32)
            for t, i in ((gam_t, 0), (bet_t, 1), (bia_t, 2)):
                a = gbb_d[i, :]
                src_ap = bass.AP(a.tensor, a.offset, [[0, P], [1, DH]])
                nc.gpsimd.dma_start(out=t[:], in_=src_ap)
            eps_t = const.tile([P, 1], F32)
            nc.vector.memset(eps_t[:], cfg.LN_EPS)
            sent_t = const.tile([2, ROW], BF16_DT)
            nc.sync.dma_start(out=sent_t[:], in_=sent_d[:])

            # ---- phase 1: htab[:, 0:136] = x @ W_ext ----
            nc.gpsimd.dma_start(out=htab0[S0:S0 + 1, 0:ROW], in_=sent_t[0:1, :])
            nc.gpsimd.dma_start(out=htab1[N1:N1 + 1, 0:ROW], in_=sent_t[1:2, :])
            with (
                tc.tile_pool(name="xp", bufs=2) as xp,
                tc.tile_pool(name="stg", bufs=2) as stg,
                tc.tile_pool(name="ps1", bufs=2, space="PSUM") as ps1,
            ):
                for g in range(NG1):
                    xk = xp.tile([P, KD, G1 * P], BF16_DT)
                    for k in range(KD):
                        nc.sync.dma_start(
                            out=xk[:, k, :],
                            in_=xT_d[k * P:(k + 1) * P,
                                     g * G1 * P:(g + 1) * G1 * P])
                    ps = ps1.tile([P, BPG, 512], F32, tag="ps1")
                    for i in range(G1):
                        pslice = ps[:, i // 3, (i % 3) * ROW:(i % 3 + 1) * ROW]
                        for k in range(KD):
                            nc.tensor.matmul(
                                pslice, lhsT=xk[:, k, i * P:(i + 1) * P],
                                rhs=wt[:, k, :],
                                start=(k == 0), stop=(k == KD - 1))
                    stage = stg.tile([P, G1, ROW], BF16_DT, tag="stage")
                    nc.scalar.copy(
                        out=stage[:].rearrange("p (b t) r -> p b t r", t=3),
                        in_=ps[:, :, 0:3 * ROW].rearrange(
                            "p b (t r) -> p b t r", r=ROW))
                    n0 = g * G1 * P
                    n1 = n0 + G1 * P
                    if n1 <= S0 or n0 >= S0:
                        t, o = (htab0, n0) if n1 <= S0 else (htab1, n0 - S0)
                        dst_ap = t[o:o + G1 * P, 0:ROW].rearrange(
                            "(b p) r -> p b r", p=P)
                        nc.gpsimd.dma_start(out=dst_ap, in_=stage[:])
                    else:
                        bs = (S0 - n0) // P   # boundary is 128-aligned
                        dst_ap = htab0[n0:S0, 0:ROW].rearrange(
                            "(b p) r -> p b r", p=P)
                        nc.gpsimd.dma_start(out=dst_ap, in_=stage[:, 0:bs, :])
                        dst_ap = htab1[0:n1 - S0, 0:ROW].rearrange(
                            "(b p) r -> p b r", p=P)
                        nc.gpsimd.dma_start(out=dst_ap, in_=stage[:, bs:, :])

            if dbg == "phase1":
                with tc.tile_pool(name="dbgp", bufs=2) as dbgp:
                    for w in range(NWIN):
                        t = dbgp.tile([P, DH], F32, tag="dbg")
                        nc.gpsimd.dma_start(out=t[:],
                                            in_=htab0[w * P:(w + 1) * P, 0:DH])
                        nc.gpsimd.dma_start(out=y_d[w * P:(w + 1) * P, :],
                                            in_=t[:])
            if dbg is None:
              # ---- phase 2 ----
              with (
                  tc.tile_pool(name="rp", bufs=2) as rp,
                  tc.tile_pool(name="wp", bufs=2) as wp,
                  tc.tile_pool(name="fp", bufs=2) as fp,
                  tc.tile_pool(name="outp", bufs=2) as outp,
              ):
                  ost = None
                  for w in range(NWIN):
                      K0, K1 = K0s[w], K1s[w]
                      KW = KWs[w]
                      off = offs[w]
                      g_main = mp.tile([P, KWMAX, ROW_T], BF16_DT, tag="gm")
                      # gathers: one span per half, split at GSUB chunks
                      for base, kn, htb, span in ((0, K0, htab0, S0 + P),
                                                  (K0, K1, htab1, N1 + P)):
                          k = 0
                          while k < kn:
                              ke = min(k + cfg.GSUB, kn)
                              a = base + k
                              b = base + ke
                              nc.gpsimd.dma_gather(
                                  out_ap=g_main[:, a:b, :],
                                  in_ap=htb[0:span, :],
                                  idxs_ap=si_t[:, (off + a) * 8:(off + b) * 8],
                                  num_idxs=(b - a) * P,
                                  num_idxs_reg=(b - a) * P,
                                  elem_size=ROW_T,
                                  queue_num=next_q())
                              k = ke

                      # a_dst for the window's nodes: self-loop slot is chunk 0
                      # of the node's own half (blend by m0 mask)
                      adw = wp.tile([P, H], F32, tag="adw")
                      if K0 and K1:
                          nc.vector.tensor_scalar(
                              out=adw[:], in0=g_main[:, 0, DH + H:DH + 2 * H],
                              scalar1=m0_t[:, 2 * w:2 * w + 1], scalar2=None,
                              op0=mybir.AluOpType.mult)
                          ad1 = wp.tile([P, H], F32, tag="ad1")
                          nc.vector.tensor_scalar(
                              out=ad1[:],
                              in0=g_main[:, K0, DH + H:DH + 2 * H],
                              scalar1=m0_t[:, 2 * w + 1:2 * w + 2],
                              scalar2=None, op0=mybir.AluOpType.mult)
                          nc.vector.tensor_tensor(
                              out=adw[:], in0=adw[:], in1=ad1[:],
                              op=mybir.AluOpType.add)
                      else:
                          nc.vector.tensor_copy(
                              out=adw[:],
                              in_=g_main[:, 0, DH + H:DH + 2 * H])

                      # scores: sc = a_src + a_dst (bcast over chunks)
                      sc = wp.tile([P, KWMAX, H], F32, tag="sc")
                      a = adw[:]
                      ad_b = bass.AP(a.tensor, a.offset,
                                     [a.ap[0], [0, KW], a.ap[1]])
                      nc.vector.tensor_tensor(
                          out=sc[:, :KW, :], in0=g_main[:, :KW, DH:DH + H],
                          in1=ad_b, op=mybir.AluOpType.add)
                      sc2 = wp.tile([P, KWMAX, H], F32, tag="sc2")
                      nc.vector.tensor_scalar_mul(out=sc2[:, :KW, :],
                                                  in0=sc[:, :KW, :],
                                                  scalar1=cfg.NEG)
                      nc.vector.tensor_tensor(out=sc2[:, :KW, :],
                                              in0=sc[:, :KW, :],
                                              in1=sc2[:, :KW, :],
                                              op=mybir.AluOpType.max)
                      wf = wp.tile([P, KWMAX, H], BF16_DT, tag="wf")
                      nc.scalar.activation(out=wf[:, :KW, :], in_=sc2[:, :KW, :],
                                           func=mybir.ActivationFunctionType.Exp)

                      # wrep = w broadcast to feature width (scalar engine, so
                      # the big DVE multiply below gets contiguous operands)
                      wrep = rp.tile([P, KWMAX, DH], BF16_DT, tag="wrep")
                      a2 = wf[:, :KW, :]
                      w_b = bass.AP(a2.tensor, a2.offset,
                                    [a2.ap[0], a2.ap[1], a2.ap[2], [0, C]])
                      nc.scalar.copy(
                          out=wrep[:, :KW, :].rearrange("p k (h c) -> p k h c",
                                                        h=H),
                          in_=w_b)
                      # rhs = h * w
                      rhs = rp.tile([P, KWMAX, DH], BF16_DT, tag="rhs")
                      nc.vector.tensor_tensor(
                          out=rhs[:, :KW, :], in0=g_main[:, :KW, 0:DH],
                          in1=wrep[:, :KW, :], op=mybir.AluOpType.mult)

                      # num = sum_k rhs: halving tree, contiguous operands
                      num = fp.tile([P, DH], F32, tag="num")
                      n = KW
                      while n > 2:
                          hh = n // 2          # fold tail onto head
                          ce = n - hh          # ceil
                          nc.vector.tensor_tensor(
                              out=rhs[:, :hh, :], in0=rhs[:, :hh, :],
                              in1=rhs[:, ce:ce + hh, :],
                              op=mybir.AluOpType.add)
                          n = ce
                      if n == 2:
                          nc.vector.tensor_tensor(
                              out=num[:], in0=rhs[:, 0, :], in1=rhs[:, 1, :],
                              op=mybir.AluOpType.add)
                      else:
                          nc.vector.tensor_copy(out=num[:], in_=rhs[:, 0, :])
                      # den = sum_k w (small strided reduce)
                      den = fp.tile([P, H], F32, tag="den")
                      nc.vector.tensor_reduce(
                          out=den[:],
                          in_=wf[:, :KW, :].rearrange("p k h -> p h k"),
                          axis=mybir.AxisListType.X, op=mybir.AluOpType.add)

                      nc.vector.tensor_scalar_add(out=den[:], in0=den[:],
                                                  scalar1=cfg.DEN_EPS)
                      nc.vector.reciprocal(out=den[:], in_=den[:])
                      y = fp.tile([P, DH], F32, tag="y")
                      da = den[:]
                      den_b = bass.AP(da.tensor, da.offset,
                                      [da.ap[0], da.ap[1], [0, C]])
                      nc.vector.tensor_tensor(
                          out=y[:].rearrange("p (h c) -> p h c", h=H),
                          in0=num[:].rearrange("p (h c) -> p h c", h=H),
                          in1=den_b, op=mybir.AluOpType.mult)
                      nc.vector.tensor_tensor(out=y[:], in0=y[:], in1=bia_t[:],
                                              op=mybir.AluOpType.add)
                      st = fp.tile([P, 6], F32, tag="st")
                      nc.vector.bn_stats(out=st[:], in_=y[:])
                      mv = fp.tile([P, 2], F32, tag="mv")
                      nc.vector.bn_aggr(out=mv[:], in_=st[:])
                      nc.scalar.activation(out=mv[:, 1:2], in_=mv[:, 1:2],
                                           func=mybir.ActivationFunctionType.Sqrt,
                                           bias=eps_t[:])
                      nc.vector.reciprocal(out=mv[:, 1:2], in_=mv[:, 1:2])
                      z = fp.tile([P, DH], F32, tag="z")
                      nc.vector.tensor_scalar(out=z[:], in0=y[:],
                                              scalar1=mv[:, 0:1],
                                              scalar2=mv[:, 1:2],
                                              op0=mybir.AluOpType.subtract,
                                              op1=mybir.AluOpType.mult)
                      nc.vector.tensor_tensor(out=z[:], in0=z[:], in1=gam_t[:],
                                              op=mybir.AluOpType.mult)
                      nc.vector.tensor_tensor(out=z[:], in0=z[:], in1=bet_t[:],
                                              op=mybir.AluOpType.add)
                      zm = fp.tile([P, DH], F32, tag="zm")
                      nc.vector.tensor_scalar(out=zm[:], in0=z[:],
                                              scalar1=0.0, scalar2=-1.0,
                                              op0=mybir.AluOpType.max,
                                              op1=mybir.AluOpType.add)
                      zn = fp.tile([P, DH], F32, tag="zn")
                      nc.vector.tensor_scalar(out=zn[:], in0=z[:], scalar1=0.0,
                                              scalar2=None,
                                              op0=mybir.AluOpType.min)
                      te = fp.tile([P, DH], F32, tag="te")
                      nc.scalar.activation(out=te[:], in_=zn[:],
                                           func=mybir.ActivationFunctionType.Exp)
                      if w % WOUT == 0:
                          ost = outp.tile([P, WOUT, DH], F32, tag="ost")
                      nc.vector.tensor_tensor(out=ost[:, w % WOUT, :], in0=zm[:],
                                              in1=te[:], op=mybir.AluOpType.add)
                      if w % WOUT == WOUT - 1 or w == NWIN - 1:
                          w0 = (w // WOUT) * WOUT
                          nb = w - w0 + 1
                          dst_ap = y_d[w0 * P:(w + 1) * P, :].rearrange(
                              "(b p) r -> p b r", p=P)
                          nc.scalar.dma_start(out=dst_ap, in_=ost[:, :nb, :])

    nc.compile()
    return nc


# --------------------------------------------------------------------------
# Entry point
# --------------------------------------------------------------------------

_CACHE = {}


def kernel(x, edge_index, W, att_src, att_dst, bias, ln_gamma, ln_beta,
           cfg=DEFAULT_CFG, trace=False, dbg=None):
    in_maps, K0s, K1s, TOTCH, out_perms = host_prep(
        cfg, x, edge_index, W, att_src, att_dst, bias, ln_gamma, ln_beta)
    key = (cfg.N, cfg.E, K0s, K1s, TOTCH, dbg)
    if key not in _CACHE:
        _CACHE[key] = build_nc(cfg, K0s, K1s, TOTCH, dbg=dbg)
    nc = _CACHE[key]
    r = run_bass_kernel_spmd(nc, in_maps, core_ids=list(range(cfg.NCORES)),
                             trace=trace)
    out = np.empty((cfg.N, cfg.DH), np.float32)
    for c in range(cfg.NCORES):
        out[out_perms[c]] = r.results[c]["y"][:cfg.D_PER_CORE]
    kernel.last_result = r
    return out
